# revision 1
# baseline (speedup 1.0000x reference)
"""KNN column-imputation kernel (nn_ColProcessor) for 8 Trainium2 cores.

Strategy: shard the 4096 query rows across 8 cores (512 rows each, data
parallel - rows are fully independent). Each core streams its [512, 16384]
slice of dist_chunk through SBUF in [128, 16384] tiles:

  - DMA: 4 column-chunks per tile (keeps the DMA->negate->select chain short)
  - ScalarE: negate in place per chunk (nd = -d), off the vector engine
  - VectorE: native top-8 selection, split into 4 column-quarters so the
    vector engine starts before the whole tile lands and finishes on a
    quarter-width scan: per quarter `max` (top-8, descending) -> combined
    `max` over the 4x8 candidates = exact global top-8 values -> per quarter
    `max_index` (first-occurrence indices, duplicate-aware sequential
    matching identical to jax.lax.top_k's lowest-index-first tie-break;
    values absent from a quarter return 0xFFFFFFFF)

The device returns, per query row, the global top-8 values and the per
quarter candidate indices. Host merge: for each of the 8 slots take the
first matching quarter (quarter order == index order, so equal-value
collisions resolve to the lowest index exactly like the reference). Rows
where the top-5 values contain duplicates need multiset bookkeeping and
fall back to an exact numpy replay; so do rows with fewer than 5 donors
among the top-8 raw candidates (donor prob ~0.9 so both are a handful of
rows). No donor-mask pass is needed over the 256MB stream, and the vector
engine stays at its 2-pass floor, which is the hard bottleneck: DMA streams
32MB/core in ~75us while the two 16384-wide selection passes cost ~137us of
vector-engine time; measured end-to-end span is ~145-150us per execution.
"""

import sys

sys.path.insert(0, "/opt/trn_rl_repo")

import numpy as np

import concourse.bacc as bacc
import concourse.mybir as mybir
from concourse.tile import TileContext

N_Q, N_FIT, N_FEAT = 4096, 16384, 32
COL, K = 3, 5
BIG = 1.0e30
NAN_FILL = 1.0e10
N_CORES = 8
ROWS = N_Q // N_CORES  # 512 query rows per core
P = 128
N_TILES = ROWS // P  # 4
NSPLIT = 4  # column-quarters for the vector-engine selection
NCHUNK = 4  # column-chunks for DMA + negate
SENT = np.uint32(0xFFFFFFFF)

_EXEC_CACHE = {}


def _build(reps=1, loop_n=None):
    """Build the per-core NEFF. loop_n wraps the body in an on-device For_i
    loop (used only for timing: the NEFF size is loop-bound independent, so
    wall-clock slopes between loop counts isolate pure HW execution time)."""
    import contextlib

    nc = bacc.Bacc("TRN2", target_bir_lowering=False)
    d_in = nc.dram_tensor("d", [ROWS, N_FIT], mybir.dt.float32, kind="ExternalInput")
    if loop_n:
        # timing-only builds take a per-call-unique salt so the axon relay's
        # identical-execution cache can't elide repeated timed runs
        salt_in = nc.dram_tensor("salt", [1, 8], mybir.dt.float32, kind="ExternalInput")
    i_out = nc.dram_tensor(
        "idx", [ROWS, 8 * NSPLIT], mybir.dt.uint32, kind="ExternalOutput"
    )
    v_out = nc.dram_tensor("vals", [ROWS, 8], mybir.dt.float32, kind="ExternalOutput")
    W = N_FIT // NCHUNK
    S = N_FIT // NSPLIT

    with TileContext(nc) as tc:
        with (
            tc.tile_pool(name="work", bufs=2) as work,
            tc.tile_pool(name="small", bufs=4) as small,
        ):
            if loop_n:
                salt_t = small.tile([1, 8], mybir.dt.float32)
                nc.sync.dma_start(out=salt_t, in_=salt_in[:, :])
            loop = tc.For_i(0, loop_n, 1) if loop_n else contextlib.nullcontext()
            with loop:
                for t in range(N_TILES * reps):
                    t = t % N_TILES
                    rs = slice(t * P, (t + 1) * P)
                    dt = work.tile([P, N_FIT], mybir.dt.float32)
                    for c in range(NCHUNK):
                        cs = slice(c * W, (c + 1) * W)
                        nc.sync.dma_start(out=dt[:, cs], in_=d_in[rs, cs])
                        nc.scalar.mul(out=dt[:, cs], in_=dt[:, cs], mul=-1.0)
                    vv = small.tile([P, 8 * NSPLIT], mybir.dt.float32)
                    v8 = small.tile([P, 8], mybir.dt.float32)
                    i8 = small.tile([P, 8 * NSPLIT], mybir.dt.uint32)
                    for s in range(NSPLIT):
                        nc.vector.max(
                            out=vv[:, 8 * s : 8 * (s + 1)], in_=dt[:, s * S : (s + 1) * S]
                        )
                    nc.vector.max(out=v8, in_=vv)
                    for s in range(NSPLIT):
                        nc.vector.max_index(
                            out=i8[:, 8 * s : 8 * (s + 1)],
                            in_max=v8,
                            in_values=dt[:, s * S : (s + 1) * S],
                        )
                    nc.sync.dma_start(out=i_out[rs, :], in_=i8)
                    nc.sync.dma_start(out=v_out[rs, :], in_=v8)
    nc.finalize()
    return nc


def _get_exec(nc):
    """Cached jitted 8-core executor for a finalized Bass module.

    Mirrors bass2jax.run_bass_via_pjrt's multi-core path but memoizes the
    jitted function so repeated calls don't re-trace/re-compile, and accepts
    already-device-resident concat inputs.
    """
    key = id(nc)
    if key in _EXEC_CACHE:
        return _EXEC_CACHE[key]

    import jax
    from jax.sharding import Mesh, PartitionSpec
    from jax.experimental.shard_map import shard_map
    from concourse import bass2jax
    from concourse import mybir as _mybir

    bass2jax.install_neuronx_cc_hook()

    partition_name = nc.partition_id_tensor.name if nc.partition_id_tensor else None
    in_names, out_names, out_avals, zero_outs = [], [], [], []
    for alloc in nc.m.functions[0].allocations:
        if not isinstance(alloc, _mybir.MemoryLocationSet):
            continue
        name = alloc.memorylocations[0].name
        if alloc.kind == "ExternalInput":
            if name != partition_name:
                in_names.append(name)
        elif alloc.kind == "ExternalOutput":
            out_names.append(name)
            shape = tuple(alloc.tensor_shape)
            dtype = _mybir.dt.np(alloc.dtype)
            out_avals.append(jax.core.ShapedArray(shape, dtype))
            zero_outs.append(np.zeros(shape, dtype))
    n_params = len(in_names)
    n_outs = len(out_avals)
    all_in_names = list(in_names) + list(out_names)
    if partition_name is not None:
        all_in_names.append(partition_name)
    donate = tuple(range(n_params, n_params + n_outs))

    def _body(*args):
        operands = list(args)
        if partition_name is not None:
            operands.append(bass2jax.partition_id_tensor())
        outs = bass2jax._bass_exec_p.bind(
            *operands,
            out_avals=tuple(out_avals),
            in_names=tuple(all_in_names),
            out_names=tuple(out_names),
            lowering_input_output_aliases=(),
            sim_require_finite=True,
            sim_require_nnan=True,
            nc=nc,
        )
        return tuple(outs)

    devices = jax.devices()[:N_CORES]
    mesh = Mesh(np.asarray(devices), ("core",))
    in_specs = (PartitionSpec("core"),) * (n_params + n_outs)
    out_specs = (PartitionSpec("core"),) * n_outs
    jitted = jax.jit(
        shard_map(
            _body, mesh=mesh, in_specs=in_specs, out_specs=out_specs, check_rep=False
        ),
        donate_argnums=donate,
        keep_unused=True,
    )

    def run(concat_inputs):
        """concat_inputs: dict name -> (N_CORES*per_core_rows, ...) array."""
        args = [concat_inputs[n] for n in in_names]
        zeros = [
            np.zeros((N_CORES * z.shape[0], *z.shape[1:]), z.dtype) for z in zero_outs
        ]
        outs = jitted(*args, *zeros)
        return {n: outs[i] for i, n in enumerate(out_names)}

    _EXEC_CACHE[key] = run
    return run


_NC = None


def _device_top8(d):
    """d: [N_Q, N_FIT] f32 -> (idx8 [N_Q, 8] int64, dup_rows [N_Q] bool).

    idx8 holds the exact raw (unmasked) top-8 smallest-distance indices in
    jax.lax.top_k order for rows without duplicate values in the top 5;
    dup_rows flags rows that need the exact host fallback.
    """
    global _NC
    if _NC is None:
        _NC = _build()
    run = _get_exec(_NC)
    out = run({"d": np.ascontiguousarray(d)})
    cand = np.asarray(out["idx"])  # [N_Q, 8*NSPLIT] u32, per-quarter candidates
    v8 = np.asarray(out["vals"])  # [N_Q, 8] f32, descending -d

    S = N_FIT // NSPLIT
    merged = np.full((d.shape[0], 8), -1, dtype=np.int64)
    for s in range(NSPLIT - 1, -1, -1):
        c = cand[:, 8 * s : 8 * (s + 1)]
        hit = c != SENT
        merged = np.where(hit, c.astype(np.int64) + s * S, merged)
    dup_rows = (v8[:, 1:K] == v8[:, : K - 1]).any(axis=1)
    return merged, dup_rows


def _exact_rows(d_rows, donor_ok, mask_fit_col, fitcol):
    """Exact numpy replay of the reference for a few rows: returns val[n]."""
    dm = np.where(
        donor_ok[None, :],
        np.where(np.isnan(d_rows), np.float32(NAN_FILL), d_rows),
        np.float32(BIG),
    )
    all_nan = np.all(np.isnan(d_rows) | ~donor_ok[None, :], axis=1)
    order = np.argsort(dm, axis=1, kind="stable")[:, :K]
    w = 1.0 - mask_fit_col[order].astype(np.float32)
    donors = fitcol[order]
    wsum = w.sum(axis=1)
    div = np.where(wsum == 0, np.float32(1.0), wsum)
    knn_val = (donors * w).sum(axis=1) / div
    obs = ~mask_fit_col
    msum = obs.sum(dtype=np.float32)
    col_sum = np.where(obs, fitcol, 0.0).sum(dtype=np.float32)
    col_mean = col_sum / (msum if msum > 0 else np.float32(1.0))
    return np.where(all_nan, col_mean, knn_val).astype(np.float32)


def kernel(
    X,
    dist_chunk,
    non_missing_fix_X,
    mask_fit_X,
    dist_idx_map,
    mask,
    row_missing_idx,
    _fit_X,
):
    X = np.asarray(X, dtype=np.float32)
    dist_chunk = np.asarray(dist_chunk, dtype=np.float32)
    non_missing_fix_X = np.asarray(non_missing_fix_X, dtype=bool)
    mask_fit_X = np.asarray(mask_fit_X, dtype=bool)
    mask = np.asarray(mask, dtype=bool)
    _fit_X = np.asarray(_fit_X, dtype=np.float32)
    rmi = np.asarray(row_missing_idx, dtype=np.int64)
    dmap = np.asarray(dist_idx_map, dtype=np.int64)

    gather_rows = dmap[rmi]
    if gather_rows.shape[0] == N_Q and np.array_equal(
        gather_rows, np.arange(N_Q, dtype=np.int64)
    ):
        d = dist_chunk
    else:
        d = np.ascontiguousarray(dist_chunk[gather_rows])
    assert d.shape == (N_Q, N_FIT)

    idx8, dup_rows = _device_top8(d)

    donor_ok = non_missing_fix_X[:, COL]
    fitcol = _fit_X[:, COL]
    mask_fit_col = mask_fit_X[:, COL]

    donor8 = donor_ok[idx8]
    cnt = donor8.sum(axis=1)
    bad_rows = dup_rows | (cnt < K)

    # first K donor slots, preserving (value, index) candidate order
    key = (~donor8) * 8 + np.arange(8)[None, :]
    sel = np.argsort(key, axis=1, kind="stable")[:, :K]
    idx5 = np.take_along_axis(idx8, sel, axis=1)

    w = 1.0 - mask_fit_col[idx5].astype(np.float32)
    donors = fitcol[idx5]
    wsum = w.sum(axis=1)
    div = np.where(wsum == 0, np.float32(1.0), wsum)
    val = (donors * w).sum(axis=1) / div

    if bad_rows.any():
        bad = np.flatnonzero(bad_rows)
        val[bad] = _exact_rows(d[bad], donor_ok, mask_fit_col, fitcol)

    col_mask = mask[rmi, COL]
    new_col = np.where(col_mask, val, X[rmi, COL]).astype(np.float32)
    out = X.copy()
    out[rmi, COL] = new_col
    return out



# revision 4
# speedup vs baseline: 1.4703x; 1.4703x over previous
"""KNN column-imputation kernel (nn_ColProcessor) for 8 Trainium2 cores.

Strategy: shard the 4096 query rows across 8 cores (512 rows each, data
parallel). The host uploads, per core, a [512, 16384] bf16 stream of
NEGATED distances with non-donor columns pushed to -2.0 (outside the
(-1, 0] range of real negated distances). Each core processes its rows
in four [128, 16384] tiles:

  - DMA: 4 column-chunks per tile.
  - DVE fold: within each 64-wide block, a 5-level pairwise tensor_max
    tree (bf16 packed -> 2x_1p DVE mode, 2 elem/cycle) plus one strided
    1x level reduces the tile to [128, 256] block maxima. This replaces
    the 1x full-width max8 scan of the old kernel: ~8.2k cycles/tile
    instead of ~33k.
  - max8 + max_index on the 256 block maxima give the top-8 blocks per
    row (top-k elements are always contained in the top-k blocks by
    block max).
  - The 8 winning 64-element blocks are fetched by an indirect DMA
    gather (per-partition block ids computed on-device) and the exact
    top-8 of that 512-element union is taken with max8 + max_index.

The host decodes block ids + in-union positions into global fit indices
and averages _fit_X[idx, COL] over the first 5 slots (donor weights are
identically 1 because the stream is donor-masked). Rows where bf16
rounding makes the selection ambiguous - a tie at the top-5 boundary
(w[4] == w[5]), a top-5 value that does not strictly beat the 8th block
score (block-selection safety margin), or fewer than 5 donors in the
union - fall back to an exact numpy replay of the reference (~1-3% of
rows). bf16 rounding is monotone, so for all other rows the selected
donor SET provably equals the exact-f32 reference set.

Per-core budget: DMA ~17MB (bf16 stream + gather + outputs) ~= 50us;
DVE ~10.5k cycles/tile * 4 ~= 45us; they overlap via double buffering.
"""

import sys

sys.path.insert(0, "/opt/trn_rl_repo")

import numpy as np

import concourse.bacc as bacc
import concourse.bass as bass
import concourse.mybir as mybir
from concourse.tile import TileContext

N_Q, N_FIT, N_FEAT = 4096, 16384, 32
COL, K = 3, 5
BIG = 1.0e30
NAN_FILL = 1.0e10
N_CORES = 8
ROWS = N_Q // N_CORES  # 512 query rows per core
P = 128
N_TILES = ROWS // P  # 4
B = 64  # block width for the fold hierarchy
NB = N_FIT // B  # 256 blocks per row
NCHUNK = 4  # column-chunks for DMA
W = N_FIT // NCHUNK  # 4096 columns per chunk
NONDONOR = -2.0  # stream fill for non-donor columns (< any real -d)
J = 6  # blocks gathered per row (top-5 elements live in the top-5 blocks)

_EXEC_CACHE = {}


def _build(reps=1, loop_n=None):
    """Build the per-core NEFF. loop_n wraps the body in an on-device For_i
    loop (used only for timing: the NEFF size is loop-bound independent, so
    wall-clock slopes between loop counts isolate pure HW execution time)."""
    import contextlib

    nc = bacc.Bacc("TRN2", target_bir_lowering=False)
    d_in = nc.dram_tensor("d", [ROWS, N_FIT], mybir.dt.bfloat16, kind="ExternalInput")
    rb_in = nc.dram_tensor("rb", [ROWS, 1], mybir.dt.float32, kind="ExternalInput")
    if loop_n:
        # timing-only builds take a per-call-unique salt so the axon relay's
        # identical-execution cache can't elide repeated timed runs
        salt_in = nc.dram_tensor("salt", [1, 8], mybir.dt.float32, kind="ExternalInput")
    idx_out = nc.dram_tensor("idx", [ROWS, 8], mybir.dt.uint32, kind="ExternalOutput")
    pos_out = nc.dram_tensor("pos", [ROWS, 8], mybir.dt.uint16, kind="ExternalOutput")
    w_out = nc.dram_tensor("wv", [ROWS, 8], mybir.dt.bfloat16, kind="ExternalOutput")
    s_out = nc.dram_tensor("sv", [ROWS, 8], mybir.dt.bfloat16, kind="ExternalOutput")

    # the same DRAM stream viewed as a table of 64-element blocks for the
    # indirect gather: row r, block b -> table row r*NB + b
    table = d_in[:, :].rearrange("r (nb b) -> (r nb) b", b=B)

    with TileContext(nc) as tc:
        with (
            tc.tile_pool(name="work", bufs=2) as work,
            tc.tile_pool(name="fold", bufs=2) as fold,
            tc.tile_pool(name="small", bufs=3) as small,
        ):
            if loop_n:
                salt_t = small.tile([1, 8], mybir.dt.float32)
                nc.sync.dma_start(out=salt_t, in_=salt_in[:, :])
            loop = tc.For_i(0, loop_n, 1) if loop_n else contextlib.nullcontext()
            with loop:
                for t in range(N_TILES * reps):
                    t = t % N_TILES
                    rs = slice(t * P, (t + 1) * P)
                    dt = work.tile([P, N_FIT], mybir.dt.bfloat16)
                    dv = dt.rearrange("p (nb b) -> p nb b", b=B)  # [P, 256, 64]
                    f1 = fold.tile([P, NB, 32], mybir.dt.bfloat16)
                    nbc = NB // NCHUNK  # blocks per chunk
                    for c in range(NCHUNK):
                        cs = slice(c * W, (c + 1) * W)
                        nc.sync.dma_start(out=dt[:, cs], in_=d_in[rs, cs])
                        bs = slice(c * nbc, (c + 1) * nbc)
                        nc.vector.tensor_max(
                            f1[:, bs, :], dv[:, bs, 0:32], dv[:, bs, 32:64]
                        )
                    f2 = fold.tile([P, NB, 16], mybir.dt.bfloat16)
                    nc.vector.tensor_max(f2, f1[:, :, 0:16], f1[:, :, 16:32])
                    f3 = fold.tile([P, NB, 8], mybir.dt.bfloat16)
                    nc.vector.tensor_max(f3, f2[:, :, 0:8], f2[:, :, 8:16])
                    f4 = fold.tile([P, NB, 4], mybir.dt.bfloat16)
                    nc.vector.tensor_max(f4, f3[:, :, 0:4], f3[:, :, 4:8])
                    f5 = fold.tile([P, NB, 2], mybir.dt.bfloat16)
                    nc.vector.tensor_max(f5, f4[:, :, 0:2], f4[:, :, 2:4])
                    f6 = small.tile([P, NB], mybir.dt.bfloat16)
                    nc.vector.tensor_max(f6, f5[:, :, 0], f5[:, :, 1])

                    s8 = small.tile([P, 8], mybir.dt.bfloat16)
                    nc.vector.max(out=s8, in_=f6)
                    blk16 = small.tile([P, 8], mybir.dt.uint16)
                    nc.vector.max_index(out=blk16, in_max=s8, in_values=f6)

                    # gather table row = local_row * NB + blk, computed in f32
                    # (exact for integers < 2^24) then cast to u32
                    rbt = small.tile([P, 1], mybir.dt.float32)
                    nc.sync.dma_start(out=rbt, in_=rb_in[rs, :])
                    blkf = small.tile([P, 8], mybir.dt.float32)
                    nc.vector.tensor_copy(out=blkf, in_=blk16)
                    idxf = small.tile([P, 8], mybir.dt.float32)
                    nc.vector.tensor_scalar_add(out=idxf, in0=blkf, scalar1=rbt[:, 0:1])
                    idxu = small.tile([P, 8], mybir.dt.uint32)
                    nc.vector.tensor_copy(out=idxu, in_=idxf)

                    # one indirect DMA per slot: HW SWDGE pairs one offset per
                    # dest partition row, so [P, 1] offsets + [P, B] dest is
                    # the only layout that gathers per-partition correctly
                    gat = small.tile([P, J, B], mybir.dt.bfloat16)
                    for s in range(J):
                        nc.gpsimd.indirect_dma_start(
                            out=gat[:, s, :],
                            out_offset=None,
                            in_=table,
                            in_offset=bass.IndirectOffsetOnAxis(
                                ap=idxu[:, s : s + 1], axis=0
                            ),
                        )

                    w8 = small.tile([P, 8], mybir.dt.bfloat16)
                    nc.vector.max(out=w8, in_=gat)
                    pos8 = small.tile([P, 8], mybir.dt.uint16)
                    nc.vector.max_index(
                        out=pos8, in_max=w8, in_values=gat.rearrange("p a b -> p (a b)")
                    )

                    nc.sync.dma_start(out=idx_out[rs, :], in_=idxu)
                    nc.sync.dma_start(out=pos_out[rs, :], in_=pos8)
                    nc.sync.dma_start(out=w_out[rs, :], in_=w8)
                    nc.sync.dma_start(out=s_out[rs, :], in_=s8)
    nc.finalize()
    return nc


def _get_exec(nc):
    """Cached jitted 8-core executor for a finalized Bass module.

    Mirrors bass2jax.run_bass_via_pjrt's multi-core path but memoizes the
    jitted function so repeated calls don't re-trace/re-compile, and accepts
    already-device-resident concat inputs.
    """
    key = id(nc)
    if key in _EXEC_CACHE:
        return _EXEC_CACHE[key]

    import jax
    from jax.sharding import Mesh, PartitionSpec
    from jax.experimental.shard_map import shard_map
    from concourse import bass2jax
    from concourse import mybir as _mybir

    bass2jax.install_neuronx_cc_hook()

    partition_name = nc.partition_id_tensor.name if nc.partition_id_tensor else None
    in_names, out_names, out_avals, zero_outs = [], [], [], []
    for alloc in nc.m.functions[0].allocations:
        if not isinstance(alloc, _mybir.MemoryLocationSet):
            continue
        name = alloc.memorylocations[0].name
        if alloc.kind == "ExternalInput":
            if name != partition_name:
                in_names.append(name)
        elif alloc.kind == "ExternalOutput":
            out_names.append(name)
            shape = tuple(alloc.tensor_shape)
            dtype = _mybir.dt.np(alloc.dtype)
            out_avals.append(jax.core.ShapedArray(shape, dtype))
            zero_outs.append(np.zeros(shape, dtype))
    n_params = len(in_names)
    n_outs = len(out_avals)
    all_in_names = list(in_names) + list(out_names)
    if partition_name is not None:
        all_in_names.append(partition_name)
    donate = tuple(range(n_params, n_params + n_outs))

    def _body(*args):
        operands = list(args)
        if partition_name is not None:
            operands.append(bass2jax.partition_id_tensor())
        outs = bass2jax._bass_exec_p.bind(
            *operands,
            out_avals=tuple(out_avals),
            in_names=tuple(all_in_names),
            out_names=tuple(out_names),
            lowering_input_output_aliases=(),
            sim_require_finite=True,
            sim_require_nnan=True,
            nc=nc,
        )
        return tuple(outs)

    devices = jax.devices()[:N_CORES]
    mesh = Mesh(np.asarray(devices), ("core",))
    in_specs = (PartitionSpec("core"),) * (n_params + n_outs)
    out_specs = (PartitionSpec("core"),) * n_outs
    jitted = jax.jit(
        shard_map(
            _body, mesh=mesh, in_specs=in_specs, out_specs=out_specs, check_rep=False
        ),
        donate_argnums=donate,
        keep_unused=True,
    )

    def run(concat_inputs):
        """concat_inputs: dict name -> (N_CORES*per_core_rows, ...) array."""
        args = [concat_inputs[n] for n in in_names]
        zeros = [
            np.zeros((N_CORES * z.shape[0], *z.shape[1:]), z.dtype) for z in zero_outs
        ]
        outs = jitted(*args, *zeros)
        return {n: outs[i] for i, n in enumerate(out_names)}

    _EXEC_CACHE[key] = run
    return run


_NC = None


def _device_topk(stream_bf16, rb_all):
    """Run the fold/gather kernel on 8 cores.

    stream_bf16: [N_Q, N_FIT] bf16 (negated, donor-masked distances)
    rb_all: [N_Q, 1] f32 (local_row * NB per core-local row)
    Returns (idx u32 [N_Q,8], pos u16 [N_Q,8], wv f32 [N_Q,8], sv f32 [N_Q,8]).
    """
    global _NC
    if _NC is None:
        _NC = _build()
    run = _get_exec(_NC)
    out = run({"d": np.ascontiguousarray(stream_bf16), "rb": rb_all})
    idxu = np.asarray(out["idx"]).astype(np.int64)
    pos = np.asarray(out["pos"]).astype(np.int64)
    wv = np.asarray(out["wv"]).astype(np.float32)
    sv = np.asarray(out["sv"]).astype(np.float32)
    return idxu, pos, wv, sv


def _exact_rows(d_rows, donor_ok, mask_fit_col, fitcol):
    """Exact numpy replay of the reference for a few rows: returns val[n]."""
    dm = np.where(
        donor_ok[None, :],
        np.where(np.isnan(d_rows), np.float32(NAN_FILL), d_rows),
        np.float32(BIG),
    )
    all_nan = np.all(np.isnan(d_rows) | ~donor_ok[None, :], axis=1)
    order = np.argsort(dm, axis=1, kind="stable")[:, :K]
    w = 1.0 - mask_fit_col[order].astype(np.float32)
    donors = fitcol[order]
    wsum = w.sum(axis=1)
    div = np.where(wsum == 0, np.float32(1.0), wsum)
    knn_val = (donors * w).sum(axis=1) / div
    obs = ~mask_fit_col
    msum = obs.sum(dtype=np.float32)
    col_sum = np.where(obs, fitcol, 0.0).sum(dtype=np.float32)
    col_mean = col_sum / (msum if msum > 0 else np.float32(1.0))
    return np.where(all_nan, col_mean, knn_val).astype(np.float32)


def kernel(
    X,
    dist_chunk,
    non_missing_fix_X,
    mask_fit_X,
    dist_idx_map,
    mask,
    row_missing_idx,
    _fit_X,
):
    import ml_dtypes

    X = np.asarray(X, dtype=np.float32)
    dist_chunk = np.asarray(dist_chunk, dtype=np.float32)
    non_missing_fix_X = np.asarray(non_missing_fix_X, dtype=bool)
    mask_fit_X = np.asarray(mask_fit_X, dtype=bool)
    mask = np.asarray(mask, dtype=bool)
    _fit_X = np.asarray(_fit_X, dtype=np.float32)
    rmi = np.asarray(row_missing_idx, dtype=np.int64)
    dmap = np.asarray(dist_idx_map, dtype=np.int64)

    gather_rows = dmap[rmi]
    if gather_rows.shape[0] == N_Q and np.array_equal(
        gather_rows, np.arange(N_Q, dtype=np.int64)
    ):
        d = dist_chunk
    else:
        d = np.ascontiguousarray(dist_chunk[gather_rows])
    assert d.shape == (N_Q, N_FIT)

    donor_ok = non_missing_fix_X[:, COL]
    fitcol = _fit_X[:, COL]
    mask_fit_col = mask_fit_X[:, COL]

    # negated, donor-masked bf16 stream (monotone rounding preserves order)
    stream = np.where(donor_ok[None, :], -d, np.float32(NONDONOR)).astype(
        ml_dtypes.bfloat16
    )
    rb = (np.arange(ROWS, dtype=np.float32) * NB).reshape(ROWS, 1)
    rb_all = np.ascontiguousarray(np.tile(rb, (N_CORES, 1)))

    idxu, pos, wv, sv = _device_topk(stream, rb_all)

    local_row = np.arange(N_Q, dtype=np.int64) % ROWS
    blk = idxu - (local_row * NB)[:, None]  # [N_Q, 8] block ids in [0, NB)
    slot = pos >> 6
    off = pos & (B - 1)
    fit_idx = np.take_along_axis(blk, slot, axis=1) * B + off  # [N_Q, 8]

    # safe rows: strict top-5 boundary, 5 real donors, and the 5th value
    # strictly beats the 8th block score (block-selection safety margin)
    safe = (wv[:, 4] > wv[:, 5]) & (wv[:, 4] > -1.5) & (wv[:, 4] > sv[:, J])

    val = fitcol[fit_idx[:, :K]].mean(axis=1).astype(np.float32)

    bad = ~safe
    if bad.any():
        val[bad] = _exact_rows(d[bad], donor_ok, mask_fit_col, fitcol)

    col_mask = mask[rmi, COL]
    new_col = np.where(col_mask, val, X[rmi, COL]).astype(np.float32)
    out = X.copy()
    out[rmi, COL] = new_col
    return out


# revision 7
# speedup vs baseline: 1.5443x; 1.0503x over previous
"""KNN column-imputation kernel (nn_ColProcessor) for 8 Trainium2 cores.

Strategy: shard the 4096 query rows across 8 cores (512 rows each, data
parallel). The host uploads, per core, a [512, 16384] bf16 stream of
NEGATED distances with non-donor columns pushed to -2.0 (outside the
(-1, 0] range of real negated distances). Each core processes its rows
in four [128, 16384] tiles:

  - DMA: 4 column-chunks per tile.
  - DVE fold: within each 64-wide block, a 5-level pairwise tensor_max
    tree (bf16 packed -> 2x_1p DVE mode, 2 elem/cycle) plus one strided
    1x level reduces the tile to [128, 256] block maxima. This replaces
    the 1x full-width max8 scan of the old kernel: ~8.2k cycles/tile
    instead of ~33k.
  - max8 + max_index on the 256 block maxima give the top-8 blocks per
    row (top-k elements are always contained in the top-k blocks by
    block max).
  - The 8 winning 64-element blocks are fetched by an indirect DMA
    gather (per-partition block ids computed on-device) and the exact
    top-8 of that 512-element union is taken with max8 + max_index.

The host decodes block ids + in-union positions into global fit indices
and averages _fit_X[idx, COL] over the first 5 slots (donor weights are
identically 1 because the stream is donor-masked). Rows where bf16
rounding makes the selection ambiguous - a tie at the top-5 boundary
(w[4] == w[5]), a top-5 value that does not strictly beat the 8th block
score (block-selection safety margin), or fewer than 5 donors in the
union - fall back to an exact numpy replay of the reference (~1-3% of
rows). bf16 rounding is monotone, so for all other rows the selected
donor SET provably equals the exact-f32 reference set.

Per-core budget: DMA ~17MB (bf16 stream + gather + outputs) ~= 50us;
DVE ~10.5k cycles/tile * 4 ~= 45us; they overlap via double buffering.
"""

import sys

sys.path.insert(0, "/opt/trn_rl_repo")

import numpy as np

import concourse.bacc as bacc
import concourse.bass as bass
import concourse.mybir as mybir
from concourse.tile import TileContext

N_Q, N_FIT, N_FEAT = 4096, 16384, 32
COL, K = 3, 5
BIG = 1.0e30
NAN_FILL = 1.0e10
N_CORES = 8
ROWS = N_Q // N_CORES  # 512 query rows per core
P = 128
N_TILES = ROWS // P  # 4
B = 64  # block width for the fold hierarchy
NB = N_FIT // B  # 256 blocks per row
NCHUNK = 2  # column-chunks for DMA
W = N_FIT // NCHUNK  # 8192 columns per chunk
NONDONOR = -2.0  # stream fill for non-donor columns (< any real -d)
J = 5  # blocks gathered per row (top-5 elements live in the top-5 blocks)

_EXEC_CACHE = {}


def _build(reps=1, loop_n=None):
    """Build the per-core NEFF. loop_n wraps the body in an on-device For_i
    loop (used only for timing: the NEFF size is loop-bound independent, so
    wall-clock slopes between loop counts isolate pure HW execution time)."""
    import contextlib

    nc = bacc.Bacc("TRN2", target_bir_lowering=False)
    d_in = nc.dram_tensor("d", [ROWS, N_FIT], mybir.dt.bfloat16, kind="ExternalInput")
    rb_in = nc.dram_tensor("rb", [ROWS, 1], mybir.dt.float32, kind="ExternalInput")
    if loop_n:
        # timing-only builds take a per-call-unique salt so the axon relay's
        # identical-execution cache can't elide repeated timed runs
        salt_in = nc.dram_tensor("salt", [1, 8], mybir.dt.float32, kind="ExternalInput")
    idx_out = nc.dram_tensor("idx", [ROWS, 8], mybir.dt.uint32, kind="ExternalOutput")
    pos_out = nc.dram_tensor("pos", [ROWS, 8], mybir.dt.uint16, kind="ExternalOutput")
    w_out = nc.dram_tensor("wv", [ROWS, 8], mybir.dt.bfloat16, kind="ExternalOutput")
    s_out = nc.dram_tensor("sv", [ROWS, 8], mybir.dt.bfloat16, kind="ExternalOutput")

    # the same DRAM stream viewed as a table of 64-element blocks for the
    # indirect gather: row r, block b -> table row r*NB + b
    table = d_in[:, :].rearrange("r (nb b) -> (r nb) b", b=B)

    with TileContext(nc) as tc:
        with (
            tc.tile_pool(name="work", bufs=2) as work,
            tc.tile_pool(name="fold", bufs=2) as fold,
            tc.tile_pool(name="small", bufs=3) as small,
        ):
            if loop_n:
                salt_t = small.tile([1, 8], mybir.dt.float32)
                nc.sync.dma_start(out=salt_t, in_=salt_in[:, :])
            loop = tc.For_i(0, loop_n, 1) if loop_n else contextlib.nullcontext()
            with loop:
                for t in range(N_TILES * reps):
                    t = t % N_TILES
                    rs = slice(t * P, (t + 1) * P)
                    dt = work.tile([P, N_FIT], mybir.dt.bfloat16)
                    dv = dt.rearrange("p (nb b) -> p nb b", b=B)  # [P, 256, 64]
                    f1 = fold.tile([P, NB, 32], mybir.dt.bfloat16)
                    nbc = NB // NCHUNK  # blocks per chunk
                    dma_engines = (nc.sync, nc.scalar)
                    for c in range(NCHUNK):
                        cs = slice(c * W, (c + 1) * W)
                        dma_engines[c % 2].dma_start(out=dt[:, cs], in_=d_in[rs, cs])
                        bs = slice(c * nbc, (c + 1) * nbc)
                        nc.vector.tensor_max(
                            f1[:, bs, :], dv[:, bs, 0:32], dv[:, bs, 32:64]
                        )
                    f2 = fold.tile([P, NB, 16], mybir.dt.bfloat16)
                    nc.vector.tensor_max(f2, f1[:, :, 0:16], f1[:, :, 16:32])
                    f3 = fold.tile([P, NB, 8], mybir.dt.bfloat16)
                    nc.vector.tensor_max(f3, f2[:, :, 0:8], f2[:, :, 8:16])
                    f4 = fold.tile([P, NB, 4], mybir.dt.bfloat16)
                    nc.vector.tensor_max(f4, f3[:, :, 0:4], f3[:, :, 4:8])
                    f5 = fold.tile([P, NB, 2], mybir.dt.bfloat16)
                    nc.vector.tensor_max(f5, f4[:, :, 0:2], f4[:, :, 2:4])
                    f6 = small.tile([P, NB], mybir.dt.bfloat16)
                    nc.vector.tensor_max(f6, f5[:, :, 0], f5[:, :, 1])

                    s8 = small.tile([P, 8], mybir.dt.bfloat16)
                    nc.vector.max(out=s8, in_=f6)
                    blk16 = small.tile([P, 8], mybir.dt.uint16)
                    nc.vector.max_index(out=blk16, in_max=s8, in_values=f6)

                    # gather table row = local_row * NB + blk, computed in f32
                    # (exact for integers < 2^24) then cast to u32
                    rbt = small.tile([P, 1], mybir.dt.float32)
                    nc.gpsimd.dma_start(out=rbt, in_=rb_in[rs, :])
                    blkf = small.tile([P, 8], mybir.dt.float32)
                    nc.vector.tensor_copy(out=blkf, in_=blk16)
                    idxf = small.tile([P, 8], mybir.dt.float32)
                    nc.vector.tensor_scalar_add(out=idxf, in0=blkf, scalar1=rbt[:, 0:1])
                    idxu = small.tile([P, 8], mybir.dt.uint32)
                    nc.vector.tensor_copy(out=idxu, in_=idxf)

                    # one indirect DMA per slot: HW SWDGE pairs one offset per
                    # dest partition row, so [P, 1] offsets + [P, B] dest is
                    # the only layout that gathers per-partition correctly
                    gat = small.tile([P, J, B], mybir.dt.bfloat16)
                    for s in range(J):
                        nc.gpsimd.indirect_dma_start(
                            out=gat[:, s, :],
                            out_offset=None,
                            in_=table,
                            in_offset=bass.IndirectOffsetOnAxis(
                                ap=idxu[:, s : s + 1], axis=0
                            ),
                        )

                    w8 = small.tile([P, 8], mybir.dt.bfloat16)
                    nc.vector.max(out=w8, in_=gat)
                    pos8 = small.tile([P, 8], mybir.dt.uint16)
                    nc.vector.max_index(
                        out=pos8, in_max=w8, in_values=gat.rearrange("p a b -> p (a b)")
                    )

                    nc.gpsimd.dma_start(out=idx_out[rs, :], in_=idxu)
                    nc.gpsimd.dma_start(out=pos_out[rs, :], in_=pos8)
                    nc.gpsimd.dma_start(out=w_out[rs, :], in_=w8)
                    nc.gpsimd.dma_start(out=s_out[rs, :], in_=s8)
    nc.finalize()
    return nc


def _get_exec(nc):
    """Cached jitted 8-core executor for a finalized Bass module.

    Mirrors bass2jax.run_bass_via_pjrt's multi-core path but memoizes the
    jitted function so repeated calls don't re-trace/re-compile, and accepts
    already-device-resident concat inputs.
    """
    key = id(nc)
    if key in _EXEC_CACHE:
        return _EXEC_CACHE[key]

    import jax
    from jax.sharding import Mesh, PartitionSpec
    from jax.experimental.shard_map import shard_map
    from concourse import bass2jax
    from concourse import mybir as _mybir

    bass2jax.install_neuronx_cc_hook()

    partition_name = nc.partition_id_tensor.name if nc.partition_id_tensor else None
    in_names, out_names, out_avals, zero_outs = [], [], [], []
    for alloc in nc.m.functions[0].allocations:
        if not isinstance(alloc, _mybir.MemoryLocationSet):
            continue
        name = alloc.memorylocations[0].name
        if alloc.kind == "ExternalInput":
            if name != partition_name:
                in_names.append(name)
        elif alloc.kind == "ExternalOutput":
            out_names.append(name)
            shape = tuple(alloc.tensor_shape)
            dtype = _mybir.dt.np(alloc.dtype)
            out_avals.append(jax.core.ShapedArray(shape, dtype))
            zero_outs.append(np.zeros(shape, dtype))
    n_params = len(in_names)
    n_outs = len(out_avals)
    all_in_names = list(in_names) + list(out_names)
    if partition_name is not None:
        all_in_names.append(partition_name)
    donate = tuple(range(n_params, n_params + n_outs))

    def _body(*args):
        operands = list(args)
        if partition_name is not None:
            operands.append(bass2jax.partition_id_tensor())
        outs = bass2jax._bass_exec_p.bind(
            *operands,
            out_avals=tuple(out_avals),
            in_names=tuple(all_in_names),
            out_names=tuple(out_names),
            lowering_input_output_aliases=(),
            sim_require_finite=True,
            sim_require_nnan=True,
            nc=nc,
        )
        return tuple(outs)

    devices = jax.devices()[:N_CORES]
    mesh = Mesh(np.asarray(devices), ("core",))
    in_specs = (PartitionSpec("core"),) * (n_params + n_outs)
    out_specs = (PartitionSpec("core"),) * n_outs
    jitted = jax.jit(
        shard_map(
            _body, mesh=mesh, in_specs=in_specs, out_specs=out_specs, check_rep=False
        ),
        donate_argnums=donate,
        keep_unused=True,
    )

    def run(concat_inputs):
        """concat_inputs: dict name -> (N_CORES*per_core_rows, ...) array."""
        args = [concat_inputs[n] for n in in_names]
        zeros = [
            np.zeros((N_CORES * z.shape[0], *z.shape[1:]), z.dtype) for z in zero_outs
        ]
        outs = jitted(*args, *zeros)
        return {n: outs[i] for i, n in enumerate(out_names)}

    _EXEC_CACHE[key] = run
    return run


_NC = None


def _device_topk(stream_bf16, rb_all):
    """Run the fold/gather kernel on 8 cores.

    stream_bf16: [N_Q, N_FIT] bf16 (negated, donor-masked distances)
    rb_all: [N_Q, 1] f32 (local_row * NB per core-local row)
    Returns (idx u32 [N_Q,8], pos u16 [N_Q,8], wv f32 [N_Q,8], sv f32 [N_Q,8]).
    """
    global _NC
    if _NC is None:
        _NC = _build()
    run = _get_exec(_NC)
    out = run({"d": np.ascontiguousarray(stream_bf16), "rb": rb_all})
    idxu = np.asarray(out["idx"]).astype(np.int64)
    pos = np.asarray(out["pos"]).astype(np.int64)
    wv = np.asarray(out["wv"]).astype(np.float32)
    sv = np.asarray(out["sv"]).astype(np.float32)
    return idxu, pos, wv, sv


def _exact_rows(d_rows, donor_ok, mask_fit_col, fitcol):
    """Exact numpy replay of the reference for a few rows: returns val[n]."""
    dm = np.where(
        donor_ok[None, :],
        np.where(np.isnan(d_rows), np.float32(NAN_FILL), d_rows),
        np.float32(BIG),
    )
    all_nan = np.all(np.isnan(d_rows) | ~donor_ok[None, :], axis=1)
    order = np.argsort(dm, axis=1, kind="stable")[:, :K]
    w = 1.0 - mask_fit_col[order].astype(np.float32)
    donors = fitcol[order]
    wsum = w.sum(axis=1)
    div = np.where(wsum == 0, np.float32(1.0), wsum)
    knn_val = (donors * w).sum(axis=1) / div
    obs = ~mask_fit_col
    msum = obs.sum(dtype=np.float32)
    col_sum = np.where(obs, fitcol, 0.0).sum(dtype=np.float32)
    col_mean = col_sum / (msum if msum > 0 else np.float32(1.0))
    return np.where(all_nan, col_mean, knn_val).astype(np.float32)


def kernel(
    X,
    dist_chunk,
    non_missing_fix_X,
    mask_fit_X,
    dist_idx_map,
    mask,
    row_missing_idx,
    _fit_X,
):
    import ml_dtypes

    X = np.asarray(X, dtype=np.float32)
    dist_chunk = np.asarray(dist_chunk, dtype=np.float32)
    non_missing_fix_X = np.asarray(non_missing_fix_X, dtype=bool)
    mask_fit_X = np.asarray(mask_fit_X, dtype=bool)
    mask = np.asarray(mask, dtype=bool)
    _fit_X = np.asarray(_fit_X, dtype=np.float32)
    rmi = np.asarray(row_missing_idx, dtype=np.int64)
    dmap = np.asarray(dist_idx_map, dtype=np.int64)

    gather_rows = dmap[rmi]
    if gather_rows.shape[0] == N_Q and np.array_equal(
        gather_rows, np.arange(N_Q, dtype=np.int64)
    ):
        d = dist_chunk
    else:
        d = np.ascontiguousarray(dist_chunk[gather_rows])
    assert d.shape == (N_Q, N_FIT)

    donor_ok = non_missing_fix_X[:, COL]
    fitcol = _fit_X[:, COL]
    mask_fit_col = mask_fit_X[:, COL]

    # negated, donor-masked bf16 stream (monotone rounding preserves order)
    stream = np.where(donor_ok[None, :], -d, np.float32(NONDONOR)).astype(
        ml_dtypes.bfloat16
    )
    rb = (np.arange(ROWS, dtype=np.float32) * NB).reshape(ROWS, 1)
    rb_all = np.ascontiguousarray(np.tile(rb, (N_CORES, 1)))

    idxu, pos, wv, sv = _device_topk(stream, rb_all)

    local_row = np.arange(N_Q, dtype=np.int64) % ROWS
    blk = idxu - (local_row * NB)[:, None]  # [N_Q, 8] block ids in [0, NB)
    slot = pos >> 6
    off = pos & (B - 1)
    fit_idx = np.take_along_axis(blk, slot, axis=1) * B + off  # [N_Q, 8]

    # safe rows: strict top-5 boundary, 5 real donors, and the 5th value
    # strictly beats the 8th block score (block-selection safety margin)
    safe = (wv[:, 4] > wv[:, 5]) & (wv[:, 4] > -1.5) & (wv[:, 4] > sv[:, J])

    val = fitcol[fit_idx[:, :K]].mean(axis=1).astype(np.float32)

    bad = ~safe
    if bad.any():
        val[bad] = _exact_rows(d[bad], donor_ok, mask_fit_col, fitcol)

    col_mask = mask[rmi, COL]
    new_col = np.where(col_mask, val, X[rmi, COL]).astype(np.float32)
    out = X.copy()
    out[rmi, COL] = new_col
    return out
